# revision 30
# baseline (speedup 1.0000x reference)
"""Trainium2 Bass kernel for nn_LocationEmbedding (GCN scatter-add + trajectory gather).

Single-launch design (8 NeuronCores, SPMD):
  - Dead-code elimination at the graph level: the output only reads
    road_embed rows for nodes appearing in (masked) trajectories (~28k of
    100k). Only edges targeting those nodes are processed on device; the
    degree normalization still uses every edge (host bincount), folded into
    per-edge weights w'_e = ef_e * dinv[row_e] * dinv[col_e].
  - Target nodes sharded by owner core (col // 12500), compacted into
    128-row blocks per core (block count = max over cores, SPMD uniform).
  - Per 128-edge chunk: one indirect row gather from the bf16 node-feature
    table (one SWDGE instruction), one fused DVE op building the weighted
    one-hot (iota == cl) * w', one PE matmul accumulating
    z_T[f, c] += ug[e, f]^T @ ohw[e, c] into PSUM.
  - Self-loops ride a static DMA (u_self input = compacted node rows) with
    a diagonal one-hot weighted dinv^2 -- no indirect gather.
  - Block tail: z_T -> SBUF bf16 (ACT), z @ W via PE (optional ones x b
    bias preload), Relu+cast on ACT into an SBUF-resident road buffer.
  - Trajectory gather: one-hot selection matmuls against the SBUF road
    blocks, accumulated per out-chunk in SBUF (no DRAM round-trip, no
    indirect gathers); host scatters packed rows into the final
    [64, 512, 128] (masked positions zero).
"""

import os
import numpy as np
import ml_dtypes

import concourse.bass as bass
import concourse.bacc as bacc
import concourse.tile as tile
from concourse import mybir
from concourse.bass_utils import run_bass_kernel_spmd

BF16 = ml_dtypes.bfloat16
P = 128
N, E, D = 100000, 1600000, 128
NCORES = 8
NS = N // NCORES          # 12500 nodes per core

F32 = mybir.dt.float32
BF = mybir.dt.bfloat16
I32 = mybir.dt.int32

LAST_EXEC_NS = None
LAST_EXEC_PARTS = None
LAST_TRACES = None


def _build_kernel(cb, nbc, j2, has_bias):
    """cb[b] = regular (gathered) chunk count for compact block b (+1 self
    chunk implicit); nbc = compact block count; j2 = out-gather chunks."""
    J = int(sum(cb)) + nbc
    nc = bacc.Bacc("TRN2", target_bir_lowering=False, debug=False)
    nf_bf = nc.dram_tensor("nf_bf", [N, P], BF, kind="ExternalInput")
    u_self = nc.dram_tensor("u_self", [nbc * P, P], BF, kind="ExternalInput")
    rows = nc.dram_tensor("rows", [P, J], I32, kind="ExternalInput")
    cl = nc.dram_tensor("cl", [P, J], F32, kind="ExternalInput")
    wch = nc.dram_tensor("wch", [P, J], F32, kind="ExternalInput")
    wt = nc.dram_tensor("wt", [P, P], BF, kind="ExternalInput")
    bvec = nc.dram_tensor("bvec", [1, P], BF, kind="ExternalInput")
    or_bc = nc.dram_tensor("or_bc", [P, j2 * P], F32, kind="ExternalInput")
    bsel = nc.dram_tensor("bsel", [P, nbc], F32, kind="ExternalInput")
    out_packed = nc.dram_tensor("out_packed", [j2 * P, P], BF, kind="ExternalOutput")

    with tile.TileContext(nc) as tc:
        with tc.tile_pool(name="sb", bufs=1) as sb, \
             tc.tile_pool(name="gp", bufs=24) as gp, \
             tc.tile_pool(name="op", bufs=24) as op_, \
             tc.tile_pool(name="blk", bufs=4) as blk, \
             tc.tile_pool(name="ps", bufs=3, space="PSUM") as ps, \
             tc.tile_pool(name="ps2", bufs=1, space="PSUM") as ps2, \
             tc.tile_pool(name="pso", bufs=4, space="PSUM") as pso:
            # iota first (no deps) so gpsimd reaches the gather stream early
            iota_i = sb.tile([P, P], I32)
            nc.gpsimd.iota(iota_i[:], pattern=[[1, P]], channel_multiplier=0)
            iota_bf = sb.tile([P, P], BF)
            nc.vector.tensor_copy(iota_bf[:], iota_i[:])
            ones_sb = sb.tile([1, P], BF)
            nc.vector.memset(ones_sb[:], 1.0)
            # per-chunk essentials first; bulky out-stage tables last
            rows_sb = sb.tile([P, J], I32)
            nc.sync.dma_start(rows_sb[:], rows[:])
            cl_sb = sb.tile([P, J], F32)
            nc.sync.dma_start(cl_sb[:], cl[:])
            w_sb = sb.tile([P, J], F32)
            nc.sync.dma_start(w_sb[:], wch[:])
            wt_sb = sb.tile([P, P], BF)
            nc.sync.dma_start(wt_sb[:], wt[:])
            b_sb = sb.tile([1, P], BF)
            nc.sync.dma_start(b_sb[:], bvec[:])
            bsel_sb = sb.tile([P, nbc], F32)
            nc.sync.dma_start(bsel_sb[:], bsel[:])
            orb_sb = sb.tile([P, j2 * P], F32)
            nc.sync.dma_start(orb_sb[:], or_bc[:])

            road_sb = sb.tile([P, nbc * P], BF)
            outsb = [sb.tile([P, P], F32, name=f"outsb{j}")
                     for j in range(j2)]

            jj = 0
            for bi in range(nbc):
                ncreg = int(cb[bi])
                nchunk = ncreg + 1
                zp = ps.tile([P, P], F32, tag="zp")
                j = 0
                while j < nchunk:
                    if j < ncreg:
                        # pair up to 2 gathers per pool tile (fewer
                        # per-instruction pool sem waits on gpsimd)
                        npair = min(2, ncreg - j)
                        ug = gp.tile([P, npair * P], BF, tag="ug")
                        for t in range(npair):
                            nc.gpsimd.indirect_dma_start(
                                out=ug[:, t * P:(t + 1) * P], out_offset=None,
                                in_=nf_bf[:],
                                in_offset=bass.IndirectOffsetOnAxis(
                                    ap=rows_sb[:, jj + t:jj + t + 1], axis=0))
                    else:
                        npair = 1
                        ug = gp.tile([P, P], BF, tag="ug")
                        nc.sync.dma_start(
                            ug[:], u_self[bi * P:(bi + 1) * P, :])
                    ohw = op_.tile([P, npair * P], BF, tag="ohw")
                    for t in range(npair):
                        nc.vector.tensor_scalar(
                            ohw[:, t * P:(t + 1) * P], iota_bf[:],
                            cl_sb[:, jj + t:jj + t + 1],
                            w_sb[:, jj + t:jj + t + 1],
                            mybir.AluOpType.is_equal, mybir.AluOpType.mult)
                        nc.tensor.matmul(
                            zp[:], lhsT=ug[:, t * P:(t + 1) * P],
                            rhs=ohw[:, t * P:(t + 1) * P],
                            start=(j + t == 0), stop=(j + t == nchunk - 1))
                    jj += npair
                    j += npair
                # tail: road[b] = relu(z @ W + b), z_T already [f, c]
                zsb = blk.tile([P, P], BF, tag="zsb")
                nc.scalar.copy(zsb[:], zp[:])
                out2 = ps2.tile([P, P], F32, tag="out2")
                if has_bias:
                    nc.tensor.matmul(out2[:], lhsT=ones_sb[:], rhs=b_sb[:],
                                     start=True, stop=False)
                nc.tensor.matmul(out2[:], lhsT=zsb[:], rhs=wt_sb[:],
                                 start=(not has_bias), stop=True)
                rslice = road_sb[:, bi * P:(bi + 1) * P]
                nc.scalar.activation(rslice, out2[:],
                                     mybir.ActivationFunctionType.Relu)
                # trajectory gather: out_j += sel_bi_j.T @ road_bi
                for j in range(j2):
                    selT = op_.tile([P, P], BF, tag="sel")
                    nc.vector.tensor_scalar(
                        selT[:], orb_sb[:, j * P:(j + 1) * P],
                        bsel_sb[:, bi:bi + 1], None,
                        mybir.AluOpType.is_equal)
                    tps = pso.tile([P, P], F32, tag="tps", bufs=4)
                    nc.tensor.matmul(tps[:], lhsT=selT[:], rhs=rslice,
                                     start=True, stop=True)
                    if bi == 0 and nbc > 1:
                        nc.vector.tensor_copy(outsb[j][:], tps[:])
                    elif bi < nbc - 1:
                        nc.vector.tensor_tensor(
                            out=outsb[j][:], in0=outsb[j][:], in1=tps[:],
                            op=mybir.AluOpType.add)
                    else:
                        # final add writes bf16 output directly -> short tail
                        osb = blk.tile([P, P], BF, tag="osb")
                        if nbc > 1:
                            nc.vector.tensor_tensor(
                                out=osb[:], in0=outsb[j][:], in1=tps[:],
                                op=mybir.AluOpType.add)
                        else:
                            nc.vector.tensor_copy(osb[:], tps[:])
                        nc.sync.dma_start(
                            out_packed[j * P:(j + 1) * P, :], osb[:])
    nc.compile()
    return nc


def kernel(**inputs):
    traj = np.asarray(inputs["traj_seqs"])[..., 0].astype(np.int64)
    seq_len = np.asarray(inputs["seq_len"]).astype(np.int64)
    nf = np.ascontiguousarray(np.asarray(inputs["node_feat"], dtype=np.float32))
    ei = np.asarray(inputs["edge_index"]).astype(np.int64)
    ef = np.asarray(inputs["edge_feat"], dtype=np.float32)
    W = np.ascontiguousarray(np.asarray(inputs["W"], dtype=np.float32))
    b = np.asarray(inputs["b"], dtype=np.float32)

    row, col = ei[0], ei[1]

    # ---------- host: normalization folded into edge weights ----------
    deg = np.bincount(col, weights=ef, minlength=N).astype(np.float32) + 1.0
    dinv = (1.0 / np.sqrt(deg)).astype(np.float32)
    nf_bf = nf.astype(BF16)

    # ---------- live target nodes (appear in masked trajectories) ----------
    flat = traj.reshape(-1)
    L = traj.shape[1]
    posmask = (np.arange(L)[None, :] < seq_len[:, None]).reshape(-1)
    live = np.unique(flat[posmask])                  # sorted global node ids
    # per-node kept-edge counts (in-degree restricted to live targets)
    live_mask = np.zeros(N, bool)
    live_mask[live] = True
    colL = col[live_mask[col]]
    ecnt = np.bincount(colL, minlength=N)
    # balanced node -> core assignment (nothing ties a target node to a col
    # range once compacted): greedy by edge count, node-capped per core
    nbc = max(1, int(np.ceil(len(live) / NCORES / P)))
    cap_nodes = nbc * P
    node_core = np.full(N, -1, np.int8)
    corder = np.argsort(-ecnt[live], kind="stable")
    core_e = np.zeros(NCORES, np.int64)
    core_n = np.zeros(NCORES, np.int64)
    for gid in live[corder]:
        cand = np.where(core_n < cap_nodes)[0]
        k = cand[np.argmin(core_e[cand])]
        node_core[gid] = k
        core_e[k] += ecnt[gid]
        core_n[k] += 1
    # compact rank per core: bin-pack nodes into nbc blocks of <=128 nodes,
    # edge-capped so padded chunk counts stay low and align across cores
    # (blocks sorted by load desc).
    node_rank = np.full(N, -1, np.int64)
    core_slots = []                    # [nbc*128] global node id per slot, -1 empty
    for k in range(NCORES):
        nk = live[node_core[live] == k]
        cap_e = int(np.ceil(ecnt[nk].sum() / (P * nbc))) * P
        cnts = ecnt[nk]
        order = np.argsort(-cnts, kind="stable")
        bin_nodes = [[] for _ in range(nbc)]
        bin_e = np.zeros(nbc, np.int64)
        bin_n = np.zeros(nbc, np.int64)
        for idx in order:
            c = int(cnts[idx])
            placed = False
            for bi in np.argsort(bin_e, kind="stable"):
                if bin_n[bi] < P and bin_e[bi] + c <= cap_e:
                    bin_nodes[bi].append(idx); bin_e[bi] += c; bin_n[bi] += 1
                    placed = True
                    break
            if not placed:
                # concentrate spill in the fullest bin so other blocks
                # keep their padded chunk count at 15
                cand = [bi for bi in range(nbc) if bin_n[bi] < P]
                bi = max(cand, key=lambda x: int(bin_e[x]))
                bin_nodes[bi].append(idx); bin_e[bi] += c; bin_n[bi] += 1
        # heaviest blocks first so spill blocks align across cores
        bo = np.argsort(-bin_e, kind="stable")
        slots = np.full(nbc * P, -1, np.int64)
        for newb, bi in enumerate(bo):
            ids = nk[bin_nodes[bi]]
            slots[newb * P:newb * P + len(ids)] = ids
            node_rank[ids] = newb * P + np.arange(len(ids))
        core_slots.append(slots)

    # ---------- edge filter + per-core layout ----------
    keep = node_rank[col] >= 0
    rowK, colK = row[keep], col[keep]
    wK = (ef[keep] * dinv[rowK] * dinv[colK]).astype(np.float32)
    crank = node_rank[colK]                          # compact col within core
    owner = node_core[colK]

    core_data = []
    for k in range(NCORES):
        m = owner == k
        ck = crank[m]
        rk = rowK[m].astype(np.int64)
        wk = wK[m]
        srt = np.lexsort((rk, ck // P))              # by block, then row
        cs, rs, ws = ck[srt], rk[srt], wk[srt]
        bcnt = np.bincount(cs // P, minlength=nbc)
        core_data.append((cs, rs, ws, bcnt))

    cb = np.zeros(nbc, np.int64)
    for k in range(NCORES):
        cb = np.maximum(cb, (core_data[k][3] + P - 1) // P)
    J = int(cb.sum()) + nbc
    cstart = np.zeros(nbc + 1, np.int64)
    np.cumsum(cb + 1, out=cstart[1:])

    oo = node_core[flat]
    sels = [np.where((oo == k) & posmask)[0] for k in range(NCORES)]
    j2 = max(1, int(np.ceil(max(len(s) for s in sels) / P)))

    has_bias = bool(np.any(b))

    in_maps = []
    for k in range(NCORES):
        cs, rs, ws, bcnt = core_data[k]
        slots = core_slots[k]
        rows_a = np.zeros((P, J), np.int32)
        cl_a = np.full((P, J), -1.0, np.float32)
        w_a = np.zeros((P, J), np.float32)
        bstart = np.zeros(nbc + 1, np.int64)
        np.cumsum(bcnt, out=bstart[1:])
        for bi in range(nbc):
            lo, hi = int(bstart[bi]), int(bstart[bi + 1])
            n = hi - lo
            nck = int(cb[bi])
            rblk = np.zeros(nck * P, np.int32)
            clblk = np.full(nck * P, -1.0, np.float32)
            wblk = np.zeros(nck * P, np.float32)
            rblk[:n] = rs[lo:hi]
            clblk[:n] = (cs[lo:hi] - bi * P).astype(np.float32)
            wblk[:n] = ws[lo:hi]
            sl = slice(int(cstart[bi]), int(cstart[bi]) + nck)
            rows_a[:, sl] = rblk.reshape(nck, P).T
            cl_a[:, sl] = clblk.reshape(nck, P).T
            w_a[:, sl] = wblk.reshape(nck, P).T
            # self chunk: diagonal over this block's live nodes, weight dinv^2
            sj = int(cstart[bi]) + nck
            bslots = slots[bi * P:(bi + 1) * P]
            filled = np.where(bslots >= 0)[0]
            cl_a[filled, sj] = filled.astype(np.float32)
            w_a[filled, sj] = dinv[bslots[filled]] ** 2

        u_self = np.zeros((nbc * P, P), BF16)
        fslots = np.where(slots >= 0)[0]
        u_self[fslots] = nf_bf[slots[fslots]]

        orows = np.zeros(j2 * P, np.float32)
        lv = node_rank[flat[sels[k]]].astype(np.float32)
        orows[:len(lv)] = lv
        or_bc = np.broadcast_to(orows[None, :], (P, j2 * P)).copy()
        bsel = (np.arange(P)[:, None] +
                P * np.arange(nbc)[None, :]).astype(np.float32)
        in_maps.append({
            "nf_bf": nf_bf, "u_self": u_self, "rows": rows_a, "cl": cl_a,
            "wch": w_a, "wt": W.astype(BF16),
            "bvec": b.astype(BF16).reshape(1, P),
            "or_bc": or_bc, "bsel": bsel,
        })

    trace = bool(os.environ.get("KERNEL_TRACE"))
    ncb = _build_kernel(cb, nbc, j2, has_bias)
    rb = run_bass_kernel_spmd(ncb, in_maps, core_ids=list(range(NCORES)),
                              trace=trace)
    global LAST_EXEC_NS, LAST_EXEC_PARTS, LAST_TRACES
    LAST_EXEC_PARTS = (rb.exec_time_ns,)
    LAST_EXEC_NS = rb.exec_time_ns
    LAST_TRACES = (rb.instructions_and_trace[1]
                   if rb.instructions_and_trace else None,)

    out = np.zeros((64 * 512, D), np.float32)
    for k in range(NCORES):
        if len(sels[k]):
            out[sels[k]] = rb.results[k]["out_packed"][:len(sels[k])].astype(np.float32)
    return out.reshape(64, 512, D)


# revision 34
# speedup vs baseline: 1.0114x; 1.0114x over previous
"""Trainium2 Bass kernel for nn_LocationEmbedding (GCN scatter-add + trajectory gather).

Single-launch design (8 NeuronCores, SPMD):
  - Dead-code elimination at the graph level: the output only reads
    road_embed rows for nodes appearing in (masked) trajectories (~28k of
    100k). Only edges targeting those nodes are processed on device; the
    degree normalization still uses every edge (host bincount), folded into
    per-edge weights w'_e = ef_e * dinv[row_e] * dinv[col_e].
  - Target nodes sharded by owner core (col // 12500), compacted into
    128-row blocks per core (block count = max over cores, SPMD uniform).
  - Per 128-edge chunk: one indirect row gather from the bf16 node-feature
    table (one SWDGE instruction), one fused DVE op building the weighted
    one-hot (iota == cl) * w', one PE matmul accumulating
    z_T[f, c] += ug[e, f]^T @ ohw[e, c] into PSUM.
  - Self-loops ride a static DMA (u_self input = compacted node rows) with
    a diagonal one-hot weighted dinv^2 -- no indirect gather.
  - Block tail: z_T -> SBUF bf16 (ACT), z @ W via PE (optional ones x b
    bias preload), Relu+cast on ACT into an SBUF-resident road buffer.
  - Trajectory gather: one-hot selection matmuls against the SBUF road
    blocks, accumulated per out-chunk in SBUF (no DRAM round-trip, no
    indirect gathers); host scatters packed rows into the final
    [64, 512, 128] (masked positions zero).
"""

import os
import numpy as np
import ml_dtypes

import concourse.bass as bass
import concourse.bacc as bacc
import concourse.tile as tile
from concourse import mybir
from concourse.bass_utils import run_bass_kernel_spmd

BF16 = ml_dtypes.bfloat16
P = 128
N, E, D = 100000, 1600000, 128
NCORES = 8
NS = N // NCORES          # 12500 nodes per core

F32 = mybir.dt.float32
BF = mybir.dt.bfloat16
I32 = mybir.dt.int32

LAST_EXEC_NS = None
LAST_EXEC_PARTS = None
LAST_TRACES = None


def _build_kernel(cb, nbc, j2, has_bias):
    """cb[b] = regular (gathered) chunk count for compact block b (+1 self
    chunk implicit); nbc = compact block count; j2 = out-gather chunks."""
    J = int(sum(cb)) + nbc
    nc = bacc.Bacc("TRN2", target_bir_lowering=False, debug=False)
    nf_bf = nc.dram_tensor("nf_bf", [N, P], BF, kind="ExternalInput")
    u_self = nc.dram_tensor("u_self", [nbc * P, P], BF, kind="ExternalInput")
    rows = nc.dram_tensor("rows", [P, J], I32, kind="ExternalInput")
    cl = nc.dram_tensor("cl", [P, J], F32, kind="ExternalInput")
    wch = nc.dram_tensor("wch", [P, J], F32, kind="ExternalInput")
    wt = nc.dram_tensor("wt", [P, P], BF, kind="ExternalInput")
    bvec = nc.dram_tensor("bvec", [1, P], BF, kind="ExternalInput")
    or_bc = nc.dram_tensor("or_bc", [P, j2 * P], F32, kind="ExternalInput")
    bsel = nc.dram_tensor("bsel", [P, nbc], F32, kind="ExternalInput")
    out_packed = nc.dram_tensor("out_packed", [P, j2 * P], BF, kind="ExternalOutput")

    with tile.TileContext(nc) as tc:
        with tc.tile_pool(name="sb", bufs=1) as sb, \
             tc.tile_pool(name="gp", bufs=24) as gp, \
             tc.tile_pool(name="op", bufs=24) as op_, \
             tc.tile_pool(name="blk", bufs=4) as blk, \
             tc.tile_pool(name="ps", bufs=3, space="PSUM") as ps, \
             tc.tile_pool(name="ps2", bufs=1, space="PSUM") as ps2, \
             tc.tile_pool(name="pso", bufs=4, space="PSUM") as pso:
            # iota first (no deps) so gpsimd reaches the gather stream early
            iota_i = sb.tile([P, P], I32)
            nc.gpsimd.iota(iota_i[:], pattern=[[1, P]], channel_multiplier=0)
            iota_bf = sb.tile([P, P], BF)
            nc.vector.tensor_copy(iota_bf[:], iota_i[:])
            ones_sb = sb.tile([1, P], BF)
            nc.vector.memset(ones_sb[:], 1.0)
            # per-chunk essentials first; bulky out-stage tables last
            rows_sb = sb.tile([P, J], I32)
            nc.sync.dma_start(rows_sb[:], rows[:])
            cl_sb = sb.tile([P, J], F32)
            nc.sync.dma_start(cl_sb[:], cl[:])
            w_sb = sb.tile([P, J], F32)
            nc.sync.dma_start(w_sb[:], wch[:])
            wt_sb = sb.tile([P, P], BF)
            nc.sync.dma_start(wt_sb[:], wt[:])
            b_sb = sb.tile([1, P], BF)
            nc.sync.dma_start(b_sb[:], bvec[:])
            bsel_sb = sb.tile([P, nbc], F32)
            nc.sync.dma_start(bsel_sb[:], bsel[:])
            orb_sb = sb.tile([P, j2 * P], F32)
            nc.sync.dma_start(orb_sb[:], or_bc[:])

            road_sb = sb.tile([P, nbc * P], BF)
            outsb = [sb.tile([P, P], F32, name=f"outsb{j}")
                     for j in range(j2)]
            out_sb = sb.tile([P, j2 * P], BF)

            jj = 0
            for bi in range(nbc):
                ncreg = int(cb[bi])
                nchunk = ncreg + 1
                zp = ps.tile([P, P], F32, tag="zp")
                j = 0
                while j < nchunk:
                    if j < ncreg:
                        # pair up to 2 gathers per pool tile (fewer
                        # per-instruction pool sem waits on gpsimd)
                        npair = min(2, ncreg - j)
                        ug = gp.tile([P, npair * P], BF, tag="ug")
                        for t in range(npair):
                            nc.gpsimd.indirect_dma_start(
                                out=ug[:, t * P:(t + 1) * P], out_offset=None,
                                in_=nf_bf[:],
                                in_offset=bass.IndirectOffsetOnAxis(
                                    ap=rows_sb[:, jj + t:jj + t + 1], axis=0))
                    else:
                        npair = 1
                        ug = gp.tile([P, P], BF, tag="ug")
                        nc.sync.dma_start(
                            ug[:], u_self[bi * P:(bi + 1) * P, :])
                    ohw = op_.tile([P, npair * P], BF, tag="ohw")
                    for t in range(npair):
                        nc.vector.tensor_scalar(
                            ohw[:, t * P:(t + 1) * P], iota_bf[:],
                            cl_sb[:, jj + t:jj + t + 1],
                            w_sb[:, jj + t:jj + t + 1],
                            mybir.AluOpType.is_equal, mybir.AluOpType.mult)
                        nc.tensor.matmul(
                            zp[:], lhsT=ug[:, t * P:(t + 1) * P],
                            rhs=ohw[:, t * P:(t + 1) * P],
                            start=(j + t == 0), stop=(j + t == nchunk - 1))
                    jj += npair
                    j += npair
                # tail: road[b] = relu(z @ W + b), z_T already [f, c]
                zsb = blk.tile([P, P], BF, tag="zsb")
                nc.scalar.copy(zsb[:], zp[:])
                out2 = ps2.tile([P, P], F32, tag="out2")
                if has_bias:
                    nc.tensor.matmul(out2[:], lhsT=ones_sb[:], rhs=b_sb[:],
                                     start=True, stop=False)
                nc.tensor.matmul(out2[:], lhsT=zsb[:], rhs=wt_sb[:],
                                 start=(not has_bias), stop=True)
                rslice = road_sb[:, bi * P:(bi + 1) * P]
                nc.scalar.activation(rslice, out2[:],
                                     mybir.ActivationFunctionType.Relu)
                # trajectory gather: out_j += sel_bi_j.T @ road_bi
                for j in range(j2):
                    selT = op_.tile([P, P], BF, tag="sel")
                    nc.vector.tensor_scalar(
                        selT[:], orb_sb[:, j * P:(j + 1) * P],
                        bsel_sb[:, bi:bi + 1], None,
                        mybir.AluOpType.is_equal)
                    tps = pso.tile([P, P], F32, tag="tps", bufs=4)
                    nc.tensor.matmul(tps[:], lhsT=selT[:], rhs=rslice,
                                     start=True, stop=True)
                    if bi == 0 and nbc > 1:
                        nc.vector.tensor_copy(outsb[j][:], tps[:])
                    elif bi < nbc - 1:
                        nc.vector.tensor_tensor(
                            out=outsb[j][:], in0=outsb[j][:], in1=tps[:],
                            op=mybir.AluOpType.add)
                    else:
                        # final add writes bf16 output slice -> one batched
                        # out DMA after the loop instead of j2 small ones
                        oslice = out_sb[:, j * P:(j + 1) * P]
                        if nbc > 1:
                            nc.vector.tensor_tensor(
                                out=oslice, in0=outsb[j][:], in1=tps[:],
                                op=mybir.AluOpType.add)
                        else:
                            nc.vector.tensor_copy(oslice, tps[:])

            nc.sync.dma_start(out_packed[:], out_sb[:])
    nc.compile()
    return nc


def kernel(**inputs):
    traj = np.asarray(inputs["traj_seqs"])[..., 0].astype(np.int64)
    seq_len = np.asarray(inputs["seq_len"]).astype(np.int64)
    nf = np.ascontiguousarray(np.asarray(inputs["node_feat"], dtype=np.float32))
    ei = np.asarray(inputs["edge_index"]).astype(np.int64)
    ef = np.asarray(inputs["edge_feat"], dtype=np.float32)
    W = np.ascontiguousarray(np.asarray(inputs["W"], dtype=np.float32))
    b = np.asarray(inputs["b"], dtype=np.float32)

    row, col = ei[0], ei[1]

    # ---------- host: normalization folded into edge weights ----------
    deg = np.bincount(col, weights=ef, minlength=N).astype(np.float32) + 1.0
    dinv = (1.0 / np.sqrt(deg)).astype(np.float32)
    nf_bf = nf.astype(BF16)

    # ---------- live target nodes (appear in masked trajectories) ----------
    flat = traj.reshape(-1)
    L = traj.shape[1]
    posmask = (np.arange(L)[None, :] < seq_len[:, None]).reshape(-1)
    live = np.unique(flat[posmask])                  # sorted global node ids
    # per-node kept-edge counts (in-degree restricted to live targets)
    live_mask = np.zeros(N, bool)
    live_mask[live] = True
    colL = col[live_mask[col]]
    ecnt = np.bincount(colL, minlength=N)
    # balanced node -> core assignment (nothing ties a target node to a col
    # range once compacted): greedy by edge count, node-capped per core
    nbc = max(1, int(np.ceil(len(live) / NCORES / P)))
    cap_nodes = nbc * P
    node_core = np.full(N, -1, np.int8)
    corder = np.argsort(-ecnt[live], kind="stable")
    core_e = np.zeros(NCORES, np.int64)
    core_n = np.zeros(NCORES, np.int64)
    for gid in live[corder]:
        cand = np.where(core_n < cap_nodes)[0]
        k = cand[np.argmin(core_e[cand])]
        node_core[gid] = k
        core_e[k] += ecnt[gid]
        core_n[k] += 1
    # compact rank per core: bin-pack nodes into nbc blocks of <=128 nodes,
    # edge-capped so padded chunk counts stay low and align across cores
    # (blocks sorted by load desc).
    node_rank = np.full(N, -1, np.int64)
    core_slots = []                    # [nbc*128] global node id per slot, -1 empty
    for k in range(NCORES):
        nk = live[node_core[live] == k]
        cap_e = int(np.ceil(ecnt[nk].sum() / (P * nbc))) * P
        cnts = ecnt[nk]
        order = np.argsort(-cnts, kind="stable")
        bin_nodes = [[] for _ in range(nbc)]
        bin_e = np.zeros(nbc, np.int64)
        bin_n = np.zeros(nbc, np.int64)
        for idx in order:
            c = int(cnts[idx])
            placed = False
            for bi in np.argsort(bin_e, kind="stable"):
                if bin_n[bi] < P and bin_e[bi] + c <= cap_e:
                    bin_nodes[bi].append(idx); bin_e[bi] += c; bin_n[bi] += 1
                    placed = True
                    break
            if not placed:
                # concentrate spill in the fullest bin so other blocks
                # keep their padded chunk count at 15
                cand = [bi for bi in range(nbc) if bin_n[bi] < P]
                bi = max(cand, key=lambda x: int(bin_e[x]))
                bin_nodes[bi].append(idx); bin_e[bi] += c; bin_n[bi] += 1
        # heaviest blocks first so spill blocks align across cores
        bo = np.argsort(-bin_e, kind="stable")
        slots = np.full(nbc * P, -1, np.int64)
        for newb, bi in enumerate(bo):
            ids = nk[bin_nodes[bi]]
            slots[newb * P:newb * P + len(ids)] = ids
            node_rank[ids] = newb * P + np.arange(len(ids))
        core_slots.append(slots)

    # ---------- edge filter + per-core layout ----------
    keep = node_rank[col] >= 0
    rowK, colK = row[keep], col[keep]
    wK = (ef[keep] * dinv[rowK] * dinv[colK]).astype(np.float32)
    crank = node_rank[colK]                          # compact col within core
    owner = node_core[colK]

    core_data = []
    for k in range(NCORES):
        m = owner == k
        ck = crank[m]
        rk = rowK[m].astype(np.int64)
        wk = wK[m]
        srt = np.lexsort((rk, ck // P))              # by block, then row
        cs, rs, ws = ck[srt], rk[srt], wk[srt]
        bcnt = np.bincount(cs // P, minlength=nbc)
        core_data.append((cs, rs, ws, bcnt))

    cb = np.zeros(nbc, np.int64)
    for k in range(NCORES):
        cb = np.maximum(cb, (core_data[k][3] + P - 1) // P)
    J = int(cb.sum()) + nbc
    cstart = np.zeros(nbc + 1, np.int64)
    np.cumsum(cb + 1, out=cstart[1:])

    oo = node_core[flat]
    sels = [np.where((oo == k) & posmask)[0] for k in range(NCORES)]
    j2 = max(1, int(np.ceil(max(len(s) for s in sels) / P)))

    has_bias = bool(np.any(b))

    in_maps = []
    for k in range(NCORES):
        cs, rs, ws, bcnt = core_data[k]
        slots = core_slots[k]
        rows_a = np.zeros((P, J), np.int32)
        cl_a = np.full((P, J), -1.0, np.float32)
        w_a = np.zeros((P, J), np.float32)
        bstart = np.zeros(nbc + 1, np.int64)
        np.cumsum(bcnt, out=bstart[1:])
        for bi in range(nbc):
            lo, hi = int(bstart[bi]), int(bstart[bi + 1])
            n = hi - lo
            nck = int(cb[bi])
            rblk = np.zeros(nck * P, np.int32)
            clblk = np.full(nck * P, -1.0, np.float32)
            wblk = np.zeros(nck * P, np.float32)
            rblk[:n] = rs[lo:hi]
            clblk[:n] = (cs[lo:hi] - bi * P).astype(np.float32)
            wblk[:n] = ws[lo:hi]
            sl = slice(int(cstart[bi]), int(cstart[bi]) + nck)
            rows_a[:, sl] = rblk.reshape(nck, P).T
            cl_a[:, sl] = clblk.reshape(nck, P).T
            w_a[:, sl] = wblk.reshape(nck, P).T
            # self chunk: diagonal over this block's live nodes, weight dinv^2
            sj = int(cstart[bi]) + nck
            bslots = slots[bi * P:(bi + 1) * P]
            filled = np.where(bslots >= 0)[0]
            cl_a[filled, sj] = filled.astype(np.float32)
            w_a[filled, sj] = dinv[bslots[filled]] ** 2

        u_self = np.zeros((nbc * P, P), BF16)
        fslots = np.where(slots >= 0)[0]
        u_self[fslots] = nf_bf[slots[fslots]]

        orows = np.zeros(j2 * P, np.float32)
        lv = node_rank[flat[sels[k]]].astype(np.float32)
        orows[:len(lv)] = lv
        or_bc = np.broadcast_to(orows[None, :], (P, j2 * P)).copy()
        bsel = (np.arange(P)[:, None] +
                P * np.arange(nbc)[None, :]).astype(np.float32)
        in_maps.append({
            "nf_bf": nf_bf, "u_self": u_self, "rows": rows_a, "cl": cl_a,
            "wch": w_a, "wt": W.astype(BF16),
            "bvec": b.astype(BF16).reshape(1, P),
            "or_bc": or_bc, "bsel": bsel,
        })

    trace = bool(os.environ.get("KERNEL_TRACE"))
    ncb = _build_kernel(cb, nbc, j2, has_bias)
    rb = run_bass_kernel_spmd(ncb, in_maps, core_ids=list(range(NCORES)),
                              trace=trace)
    global LAST_EXEC_NS, LAST_EXEC_PARTS, LAST_TRACES
    LAST_EXEC_PARTS = (rb.exec_time_ns,)
    LAST_EXEC_NS = rb.exec_time_ns
    LAST_TRACES = (rb.instructions_and_trace[1]
                   if rb.instructions_and_trace else None,)

    out = np.zeros((64 * 512, D), np.float32)
    for k in range(NCORES):
        if len(sels[k]):
            pk = rb.results[k]["out_packed"]          # [128, j2*128] bf16
            pk = pk.reshape(P, j2, P).transpose(1, 0, 2).reshape(j2 * P, P)
            out[sels[k]] = pk[:len(sels[k])].astype(np.float32)
    return out.reshape(64, 512, D)


# revision 37
# speedup vs baseline: 1.0173x; 1.0058x over previous
"""Trainium2 Bass kernel for nn_LocationEmbedding (GCN scatter-add + trajectory gather).

Single-launch design (8 NeuronCores, SPMD):
  - Dead-code elimination at the graph level: the output only reads
    road_embed rows for nodes appearing in (masked) trajectories (~28k of
    100k). Only edges targeting those nodes are processed on device; the
    degree normalization still uses every edge (host bincount), folded into
    per-edge weights w'_e = ef_e * dinv[row_e] * dinv[col_e].
  - Target nodes sharded by owner core (col // 12500), compacted into
    128-row blocks per core (block count = max over cores, SPMD uniform).
  - Per 128-edge chunk: one indirect row gather from the bf16 node-feature
    table (one SWDGE instruction), one fused DVE op building the weighted
    one-hot (iota == cl) * w', one PE matmul accumulating
    z_T[f, c] += ug[e, f]^T @ ohw[e, c] into PSUM.
  - Self-loops ride a static DMA (u_self input = compacted node rows) with
    a diagonal one-hot weighted dinv^2 -- no indirect gather.
  - Block tail: z_T -> SBUF bf16 (ACT), z @ W via PE (optional ones x b
    bias preload), Relu+cast on ACT into an SBUF-resident road buffer.
  - Trajectory gather: one-hot selection matmuls against the SBUF road
    blocks, accumulated per out-chunk in SBUF (no DRAM round-trip, no
    indirect gathers); host scatters packed rows into the final
    [64, 512, 128] (masked positions zero).
"""

import os
import numpy as np
import ml_dtypes

import concourse.bass as bass
import concourse.bacc as bacc
import concourse.tile as tile
from concourse import mybir
from concourse.bass_utils import run_bass_kernel_spmd

BF16 = ml_dtypes.bfloat16
P = 128
N, E, D = 100000, 1600000, 128
NCORES = 8
NS = N // NCORES          # 12500 nodes per core

F32 = mybir.dt.float32
BF = mybir.dt.bfloat16
I32 = mybir.dt.int32

LAST_EXEC_NS = None
LAST_EXEC_PARTS = None
LAST_TRACES = None


def _build_kernel(cb, nbc, j2, has_bias):
    """cb[b] = regular (gathered) chunk count for compact block b (+1 self
    chunk implicit); nbc = compact block count; j2 = out-gather chunks."""
    J = int(sum(cb)) + nbc
    nc = bacc.Bacc("TRN2", target_bir_lowering=False, debug=False)
    nf_bf = nc.dram_tensor("nf_bf", [N, P], BF, kind="ExternalInput")
    u_self = nc.dram_tensor("u_self", [nbc * P, P], BF, kind="ExternalInput")
    rows = nc.dram_tensor("rows", [P, J], I32, kind="ExternalInput")
    cl = nc.dram_tensor("cl", [P, J], F32, kind="ExternalInput")
    wch = nc.dram_tensor("wch", [P, J], F32, kind="ExternalInput")
    wt = nc.dram_tensor("wt", [P, P], BF, kind="ExternalInput")
    bvec = nc.dram_tensor("bvec", [1, P], BF, kind="ExternalInput")
    or_bc = nc.dram_tensor("or_bc", [P, j2 * P], F32, kind="ExternalInput")
    bsel = nc.dram_tensor("bsel", [P, nbc], F32, kind="ExternalInput")
    iotab = nc.dram_tensor("iotab", [P, P], BF, kind="ExternalInput")
    out_packed = nc.dram_tensor("out_packed", [P, j2 * P], BF, kind="ExternalOutput")

    with tile.TileContext(nc) as tc:
        with tc.tile_pool(name="sb", bufs=1) as sb, \
             tc.tile_pool(name="gp", bufs=24) as gp, \
             tc.tile_pool(name="op", bufs=24) as op_, \
             tc.tile_pool(name="blk", bufs=4) as blk, \
             tc.tile_pool(name="ps", bufs=3, space="PSUM") as ps, \
             tc.tile_pool(name="ps2", bufs=1, space="PSUM") as ps2, \
             tc.tile_pool(name="pso", bufs=4, space="PSUM") as pso:
            # iota comes in as data: keeps gpsimd free of ext-isa ops, so no
            # ~6us ucode IRAM load delays the first indirect gather
            iota_bf = sb.tile([P, P], BF)
            nc.sync.dma_start(iota_bf[:], iotab[:])
            ones_sb = sb.tile([1, P], BF)
            nc.vector.memset(ones_sb[:], 1.0)
            # per-chunk essentials first; bulky out-stage tables last
            rows_sb = sb.tile([P, J], I32)
            nc.sync.dma_start(rows_sb[:], rows[:])
            cl_sb = sb.tile([P, J], F32)
            nc.sync.dma_start(cl_sb[:], cl[:])
            w_sb = sb.tile([P, J], F32)
            nc.sync.dma_start(w_sb[:], wch[:])
            wt_sb = sb.tile([P, P], BF)
            nc.sync.dma_start(wt_sb[:], wt[:])
            b_sb = sb.tile([1, P], BF)
            nc.sync.dma_start(b_sb[:], bvec[:])
            bsel_sb = sb.tile([P, nbc], F32)
            nc.sync.dma_start(bsel_sb[:], bsel[:])
            orb_sb = sb.tile([P, j2 * P], F32)
            nc.sync.dma_start(orb_sb[:], or_bc[:])

            road_sb = sb.tile([P, nbc * P], BF)
            outsb = [sb.tile([P, P], F32, name=f"outsb{j}")
                     for j in range(j2)]
            out_sb = sb.tile([P, j2 * P], BF)

            jj = 0
            for bi in range(nbc):
                ncreg = int(cb[bi])
                nchunk = ncreg + 1
                zp = ps.tile([P, P], F32, tag="zp")
                j = 0
                while j < nchunk:
                    if j < ncreg:
                        # pair up to 2 gathers per pool tile (fewer
                        # per-instruction pool sem waits on gpsimd)
                        npair = min(2, ncreg - j)
                        ug = gp.tile([P, npair * P], BF, tag="ug")
                        for t in range(npair):
                            nc.gpsimd.indirect_dma_start(
                                out=ug[:, t * P:(t + 1) * P], out_offset=None,
                                in_=nf_bf[:],
                                in_offset=bass.IndirectOffsetOnAxis(
                                    ap=rows_sb[:, jj + t:jj + t + 1], axis=0))
                    else:
                        npair = 1
                        ug = gp.tile([P, P], BF, tag="ug")
                        nc.sync.dma_start(
                            ug[:], u_self[bi * P:(bi + 1) * P, :])
                    ohw = op_.tile([P, npair * P], BF, tag="ohw")
                    for t in range(npair):
                        nc.vector.tensor_scalar(
                            ohw[:, t * P:(t + 1) * P], iota_bf[:],
                            cl_sb[:, jj + t:jj + t + 1],
                            w_sb[:, jj + t:jj + t + 1],
                            mybir.AluOpType.is_equal, mybir.AluOpType.mult)
                        nc.tensor.matmul(
                            zp[:], lhsT=ug[:, t * P:(t + 1) * P],
                            rhs=ohw[:, t * P:(t + 1) * P],
                            start=(j + t == 0), stop=(j + t == nchunk - 1))
                    jj += npair
                    j += npair
                # tail: road[b] = relu(z @ W + b), z_T already [f, c]
                zsb = blk.tile([P, P], BF, tag="zsb")
                nc.scalar.copy(zsb[:], zp[:])
                out2 = ps2.tile([P, P], F32, tag="out2")
                if has_bias:
                    nc.tensor.matmul(out2[:], lhsT=ones_sb[:], rhs=b_sb[:],
                                     start=True, stop=False)
                nc.tensor.matmul(out2[:], lhsT=zsb[:], rhs=wt_sb[:],
                                 start=(not has_bias), stop=True)
                rslice = road_sb[:, bi * P:(bi + 1) * P]
                nc.scalar.activation(rslice, out2[:],
                                     mybir.ActivationFunctionType.Relu)
                # trajectory gather: out_j += sel_bi_j.T @ road_bi
                for j in range(j2):
                    selT = op_.tile([P, P], BF, tag="sel")
                    nc.vector.tensor_scalar(
                        selT[:], orb_sb[:, j * P:(j + 1) * P],
                        bsel_sb[:, bi:bi + 1], None,
                        mybir.AluOpType.is_equal)
                    tps = pso.tile([P, P], F32, tag="tps", bufs=4)
                    nc.tensor.matmul(tps[:], lhsT=selT[:], rhs=rslice,
                                     start=True, stop=True)
                    if bi == 0 and nbc > 1:
                        nc.vector.tensor_copy(outsb[j][:], tps[:])
                    elif bi < nbc - 1:
                        nc.vector.tensor_tensor(
                            out=outsb[j][:], in0=outsb[j][:], in1=tps[:],
                            op=mybir.AluOpType.add)
                    else:
                        # final add writes bf16 output slice -> one batched
                        # out DMA after the loop instead of j2 small ones
                        oslice = out_sb[:, j * P:(j + 1) * P]
                        if nbc > 1:
                            nc.vector.tensor_tensor(
                                out=oslice, in0=outsb[j][:], in1=tps[:],
                                op=mybir.AluOpType.add)
                        else:
                            nc.vector.tensor_copy(oslice, tps[:])

            nc.sync.dma_start(out_packed[:], out_sb[:])
    nc.compile()
    return nc


def kernel(**inputs):
    traj = np.asarray(inputs["traj_seqs"])[..., 0].astype(np.int64)
    seq_len = np.asarray(inputs["seq_len"]).astype(np.int64)
    nf = np.ascontiguousarray(np.asarray(inputs["node_feat"], dtype=np.float32))
    ei = np.asarray(inputs["edge_index"]).astype(np.int64)
    ef = np.asarray(inputs["edge_feat"], dtype=np.float32)
    W = np.ascontiguousarray(np.asarray(inputs["W"], dtype=np.float32))
    b = np.asarray(inputs["b"], dtype=np.float32)

    row, col = ei[0], ei[1]

    # ---------- host: normalization folded into edge weights ----------
    deg = np.bincount(col, weights=ef, minlength=N).astype(np.float32) + 1.0
    dinv = (1.0 / np.sqrt(deg)).astype(np.float32)
    nf_bf = nf.astype(BF16)

    # ---------- live target nodes (appear in masked trajectories) ----------
    flat = traj.reshape(-1)
    L = traj.shape[1]
    posmask = (np.arange(L)[None, :] < seq_len[:, None]).reshape(-1)
    live = np.unique(flat[posmask])                  # sorted global node ids
    # per-node kept-edge counts (in-degree restricted to live targets)
    live_mask = np.zeros(N, bool)
    live_mask[live] = True
    colL = col[live_mask[col]]
    ecnt = np.bincount(colL, minlength=N)
    # balanced node -> core assignment (nothing ties a target node to a col
    # range once compacted): greedy by edge count, node-capped per core
    nbc = max(1, int(np.ceil(len(live) / NCORES / P)))
    cap_nodes = nbc * P
    node_core = np.full(N, -1, np.int8)
    corder = np.argsort(-ecnt[live], kind="stable")
    core_e = np.zeros(NCORES, np.int64)
    core_n = np.zeros(NCORES, np.int64)
    for gid in live[corder]:
        cand = np.where(core_n < cap_nodes)[0]
        k = cand[np.argmin(core_e[cand])]
        node_core[gid] = k
        core_e[k] += ecnt[gid]
        core_n[k] += 1
    # compact rank per core: bin-pack nodes into nbc blocks of <=128 nodes,
    # edge-capped so padded chunk counts stay low and align across cores
    # (blocks sorted by load desc).
    node_rank = np.full(N, -1, np.int64)
    core_slots = []                    # [nbc*128] global node id per slot, -1 empty
    for k in range(NCORES):
        nk = live[node_core[live] == k]
        cap_e = int(np.ceil(ecnt[nk].sum() / (P * nbc))) * P
        cnts = ecnt[nk]
        order = np.argsort(-cnts, kind="stable")
        bin_nodes = [[] for _ in range(nbc)]
        bin_e = np.zeros(nbc, np.int64)
        bin_n = np.zeros(nbc, np.int64)
        for idx in order:
            c = int(cnts[idx])
            placed = False
            for bi in np.argsort(bin_e, kind="stable"):
                if bin_n[bi] < P and bin_e[bi] + c <= cap_e:
                    bin_nodes[bi].append(idx); bin_e[bi] += c; bin_n[bi] += 1
                    placed = True
                    break
            if not placed:
                # concentrate spill in the fullest bin so other blocks
                # keep their padded chunk count at 15
                cand = [bi for bi in range(nbc) if bin_n[bi] < P]
                bi = max(cand, key=lambda x: int(bin_e[x]))
                bin_nodes[bi].append(idx); bin_e[bi] += c; bin_n[bi] += 1
        # heaviest blocks first so spill blocks align across cores
        bo = np.argsort(-bin_e, kind="stable")
        slots = np.full(nbc * P, -1, np.int64)
        for newb, bi in enumerate(bo):
            ids = nk[bin_nodes[bi]]
            slots[newb * P:newb * P + len(ids)] = ids
            node_rank[ids] = newb * P + np.arange(len(ids))
        core_slots.append(slots)

    # ---------- edge filter + per-core layout ----------
    keep = node_rank[col] >= 0
    rowK, colK = row[keep], col[keep]
    wK = (ef[keep] * dinv[rowK] * dinv[colK]).astype(np.float32)
    crank = node_rank[colK]                          # compact col within core
    owner = node_core[colK]

    core_data = []
    for k in range(NCORES):
        m = owner == k
        ck = crank[m]
        rk = rowK[m].astype(np.int64)
        wk = wK[m]
        srt = np.lexsort((rk, ck // P))              # by block, then row
        cs, rs, ws = ck[srt], rk[srt], wk[srt]
        bcnt = np.bincount(cs // P, minlength=nbc)
        core_data.append((cs, rs, ws, bcnt))

    cb = np.zeros(nbc, np.int64)
    for k in range(NCORES):
        cb = np.maximum(cb, (core_data[k][3] + P - 1) // P)
    J = int(cb.sum()) + nbc
    cstart = np.zeros(nbc + 1, np.int64)
    np.cumsum(cb + 1, out=cstart[1:])

    oo = node_core[flat]
    sels = [np.where((oo == k) & posmask)[0] for k in range(NCORES)]
    j2 = max(1, int(np.ceil(max(len(s) for s in sels) / P)))

    has_bias = bool(np.any(b))

    in_maps = []
    for k in range(NCORES):
        cs, rs, ws, bcnt = core_data[k]
        slots = core_slots[k]
        rows_a = np.zeros((P, J), np.int32)
        cl_a = np.full((P, J), -1.0, np.float32)
        w_a = np.zeros((P, J), np.float32)
        bstart = np.zeros(nbc + 1, np.int64)
        np.cumsum(bcnt, out=bstart[1:])
        for bi in range(nbc):
            lo, hi = int(bstart[bi]), int(bstart[bi + 1])
            n = hi - lo
            nck = int(cb[bi])
            rblk = np.zeros(nck * P, np.int32)
            clblk = np.full(nck * P, -1.0, np.float32)
            wblk = np.zeros(nck * P, np.float32)
            rblk[:n] = rs[lo:hi]
            clblk[:n] = (cs[lo:hi] - bi * P).astype(np.float32)
            wblk[:n] = ws[lo:hi]
            sl = slice(int(cstart[bi]), int(cstart[bi]) + nck)
            rows_a[:, sl] = rblk.reshape(nck, P).T
            cl_a[:, sl] = clblk.reshape(nck, P).T
            w_a[:, sl] = wblk.reshape(nck, P).T
            # self chunk: diagonal over this block's live nodes, weight dinv^2
            sj = int(cstart[bi]) + nck
            bslots = slots[bi * P:(bi + 1) * P]
            filled = np.where(bslots >= 0)[0]
            cl_a[filled, sj] = filled.astype(np.float32)
            w_a[filled, sj] = dinv[bslots[filled]] ** 2

        u_self = np.zeros((nbc * P, P), BF16)
        fslots = np.where(slots >= 0)[0]
        u_self[fslots] = nf_bf[slots[fslots]]

        orows = np.zeros(j2 * P, np.float32)
        lv = node_rank[flat[sels[k]]].astype(np.float32)
        orows[:len(lv)] = lv
        or_bc = np.broadcast_to(orows[None, :], (P, j2 * P)).copy()
        bsel = (np.arange(P)[:, None] +
                P * np.arange(nbc)[None, :]).astype(np.float32)
        in_maps.append({
            "nf_bf": nf_bf, "u_self": u_self, "rows": rows_a, "cl": cl_a,
            "wch": w_a, "wt": W.astype(BF16),
            "bvec": b.astype(BF16).reshape(1, P),
            "or_bc": or_bc, "bsel": bsel,
            "iotab": np.tile(
                np.arange(P, dtype=np.float32).astype(BF16)[None, :], (P, 1)),
        })

    trace = bool(os.environ.get("KERNEL_TRACE"))
    ncb = _build_kernel(cb, nbc, j2, has_bias)
    rb = run_bass_kernel_spmd(ncb, in_maps, core_ids=list(range(NCORES)),
                              trace=trace)
    global LAST_EXEC_NS, LAST_EXEC_PARTS, LAST_TRACES
    LAST_EXEC_PARTS = (rb.exec_time_ns,)
    LAST_EXEC_NS = rb.exec_time_ns
    LAST_TRACES = (rb.instructions_and_trace[1]
                   if rb.instructions_and_trace else None,)

    out = np.zeros((64 * 512, D), np.float32)
    for k in range(NCORES):
        if len(sels[k]):
            pk = rb.results[k]["out_packed"]          # [128, j2*128] bf16
            pk = pk.reshape(P, j2, P).transpose(1, 0, 2).reshape(j2 * P, P)
            out[sels[k]] = pk[:len(sels[k])].astype(np.float32)
    return out.reshape(64, 512, D)
